# revision 1
# baseline (speedup 1.0000x reference)
"""AdaConv Trainium2 kernel (8 NeuronCores, batch-sharded conv with
weight-sharded kernel prediction + AllToAll exchange, bf16 compute).

Per-core layout (core n owns sample n):
  Stage A : predict per-sample depthwise (dw) / pointwise (pk) kernels.
            dk_w / pwk_w sharded across cores by output channel (1/8 each);
            every core computes its slice for ALL 8 samples, then one
            AllToAll hands core n the full kernels for sample n.
  Stats   : instance-norm statistics of predicted[n], computed from the
            SBUF-resident conv input tiles and folded algebraically into
            the conv epilogue:
              out = conv(x_raw, E) * (1/sigma) + (pb - mu/sigma * S)
            with E = pk @ dw (pointwise folded into depthwise) and
            S[oc] = sum of E over (ic, taps).
  Conv    : grouped 3x3 conv (8 groups of 64->64 ch) over reflect-padded
            input, 2 groups per matmul (block-diagonal 128x128 weights),
            bf16 tensor-engine matmuls into fp32 PSUM, ACT epilogue.
"""

import sys

if '/opt/trn_rl_repo' not in sys.path:
    sys.path.insert(0, '/opt/trn_rl_repo')

import numpy as np
import ml_dtypes

N_CORES = 8
C = 512
H = W = 128
PW = W + 2               # padded row length (130)
PHW = (H + 2) * PW       # padded channel-block image size (16900)
RPC = 3                  # output rows per psum chunk
NCH = RPC * PW           # 390
G = 8                    # conv groups
OCS = 32768 // N_CORES   # dw/pk output-channel slice per core (4096)
KM = 2048                # dw predictor contraction (512ci * 2*2)
EPS = 1e-5
M_TOT = float(C * H * W)
BLK = 10 * OCS + 512     # A2A per-block floats: 9*4096 dw + 4096 pk + 512 pb

_CACHE = {}


def _build(norm: bool):
    import concourse.bacc as bacc
    import concourse.mybir as mybir
    import concourse.tile as tile

    f32 = mybir.dt.float32
    f32r = mybir.dt.float32r
    bf16 = mybir.dt.bfloat16
    AX = mybir.AxisListType
    ALU = mybir.AluOpType
    ACTF = mybir.ActivationFunctionType

    nc = bacc.Bacc("TRN2", target_bir_lowering=False, debug=False,
                   enable_asserts=True, num_devices=N_CORES)

    # ---- DRAM parameters (per-core shards prepared on host) ----
    xin = nc.dram_tensor("xin", [C, H * W], f32, kind="ExternalInput")
    wt = nc.dram_tensor("wt", [8, 16, 128, 512], bf16, kind="ExternalInput")
    pkt = nc.dram_tensor("pkt", [8, 4, 128, 512], bf16, kind="ExternalInput")
    pbt = nc.dram_tensor("pbt", [4, 128, C], bf16, kind="ExternalInput")
    s_im = nc.dram_tensor("s_im", [16, 128, 72], bf16, kind="ExternalInput")
    st_raw = nc.dram_tensor("st_raw", [4, 128, 8, 16], f32, kind="ExternalInput")
    dkb = nc.dram_tensor("dkb", [8, 512], bf16, kind="ExternalInput")
    pkb = nc.dram_tensor("pkb", [8, 512], bf16, kind="ExternalInput")
    pwbb = nc.dram_tensor("pwbb", [128, 4], f32, kind="ExternalInput")
    ones_r = nc.dram_tensor("ones_r", [128, 128], f32r, kind="ExternalInput")
    ones_b = nc.dram_tensor("ones_b", [1, 128], bf16, kind="ExternalInput")
    zeros_e = nc.dram_tensor("zeros_e", [128, 9 * 128], bf16, kind="ExternalInput")
    out = nc.dram_tensor("out", [C, H * W], f32, kind="ExternalOutput")

    a2a_in = nc.dram_tensor("a2a_in", [N_CORES, BLK], bf16)
    a2a_out = nc.dram_tensor("a2a_out", [N_CORES, BLK], bf16)

    with tile.TileContext(nc) as tc:
        with tc.tile_pool(name="const", bufs=1) as cpool, \
             tc.tile_pool(name="xblk", bufs=1) as xpool, \
             tc.tile_pool(name="stats", bufs=1) as spool, \
             tc.tile_pool(name="pe", bufs=1) as epool, \
             tc.tile_pool(name="ps_a", bufs=1, space="PSUM") as ps_a, \
             tc.tile_pool(name="ps_c", bufs=1, space="PSUM") as ps_c:

            ones_sb = cpool.tile([128, 128], f32r)
            nc.sync.dma_start(ones_sb[:], ones_r.ap())
            onesb_sb = cpool.tile([1, 128], bf16)
            nc.sync.dma_start(onesb_sb[:], ones_b.ap())
            pb_sb = cpool.tile([128, 32], bf16)
            s_vec = cpool.tile([128, 4], f32)
            svt = cpool.tile([128, 8], f32)

            # ---------- X blocks: load all 4 resident (bf16, reflect-pad) ----
            xv2 = xin.ap().rearrange("(b p) (h w) -> b p h w", p=128, w=W)
            xts = []
            for gp in range(4):
                xt = xpool.tile([128, PHW], bf16, tag="x", bufs=4,
                                name=f"xt{gp}")
                xd = xt[:].rearrange("p (r c) -> p r c", c=PW)
                xsrc = xv2[gp]  # [128, 128, 128] f32
                nc.gpsimd.dma_start(xd[:, 1:65, 1:129], xsrc[:, 0:64, :])
                nc.gpsimd.dma_start(xd[:, 65:129, 1:129], xsrc[:, 64:128, :])
                nc.gpsimd.dma_start(xd[:, 0:1, 1:129], xsrc[:, 1:2, :])
                nc.gpsimd.dma_start(xd[:, 129:130, 1:129], xsrc[:, 126:127, :])
                # reflect columns on-chip (covers halo-row corners too)
                nc.vector.tensor_copy(xd[:, :, 0:1], xd[:, :, 2:3])
                nc.vector.tensor_copy(xd[:, :, 129:130], xd[:, :, 127:128])
                xts.append(xt)

            # ---------- stats partials from resident tiles ----------
            if norm:
                acc = spool.tile([128, 64], f32)
                scratch = spool.tile([128, 2048], bf16, tag="sq", bufs=1)
                for gp in range(4):
                    xdv = xts[gp][:].rearrange("p (r c) -> p r c", c=PW)
                    for sc in range(8):
                        i = gp * 8 + sc
                        # valid rows band: padded rows 1+16*sc .. 1+16*(sc+1)
                        sl = xdv[:, 1 + 16 * sc:1 + 16 * (sc + 1), 1:129]
                        nc.vector.tensor_reduce(acc[:, i:i + 1], sl,
                                                axis=AX.XY, op=ALU.add)
                        nc.scalar.activation(
                            scratch[:].rearrange("p (a b) -> p a b", b=128),
                            sl, ACTF.Square,
                            accum_out=acc[:, 32 + i:33 + i])

            # ================= stage A (scoped pool) ========================
            with tc.tile_pool(name="sa", bufs=1) as apool, \
                 tc.tile_pool(name="wts", bufs=1) as wpool:
                s_sb = apool.tile([128, 16 * 72], bf16)
                nc.sync.dma_start(
                    s_sb[:].rearrange("p (k c) -> p k c", k=16),
                    s_im.ap().rearrange("k p c -> p k c"))
                st_sb = apool.tile([128, 4 * 128], f32)
                nc.sync.dma_start(
                    st_sb[:].rearrange("p (b q) -> p b q", b=4),
                    st_raw.ap().rearrange("b p n q -> p b (n q)"))
                pwbb_sb = apool.tile([128, 4], f32)
                nc.sync.dma_start(pwbb_sb[:], pwbb.ap())
                pbt_sb = apool.tile([128, 4 * C], bf16)
                nc.sync.dma_start(
                    pbt_sb[:].rearrange("p (k c) -> p k c", k=4),
                    pbt.ap().rearrange("k p c -> p k c"))


                # s_d = mean over the 4x4 style map -> [ci(128) x 4, 8]
                sd_f = apool.tile([128, 32], f32)
                sd_b = apool.tile([128, 32], bf16)
                for cb in range(4):
                    nc.vector.tensor_reduce(
                        sd_f[:, cb * 8:(cb + 1) * 8],
                        st_sb[:, cb * 128:(cb + 1) * 128].rearrange(
                            "p (n q) -> p n q", q=16),
                        axis=AX.X, op=ALU.add)
                nc.vector.tensor_scalar_mul(sd_f[:], sd_f[:], 1.0 / 16.0)
                nc.vector.tensor_copy(sd_b[:], sd_f[:])

                # dw slice for all samples: [72=(n,tap), 4096]
                dw_sb = apool.tile([72, OCS], bf16)
                for nch in range(8):
                    ps_dw = ps_a.tile([72, 512], f32, tag="psa", bufs=3,
                                      name=f"psdw{nch}")
                    for half in range(2):
                        wt_sb = wpool.tile([128, 8 * 512], bf16, tag="wt",
                                           bufs=2, name=f"wt{nch}_{half}")
                        nc.sync.dma_start(
                            wt_sb[:].rearrange("p (k c) -> p k c", k=8),
                            wt.ap()[nch, half * 8:half * 8 + 8]
                            .rearrange("k p c -> p k c"))
                        for k8 in range(8):
                            kc = half * 8 + k8
                            nc.tensor.matmul(
                                ps_dw[:],
                                s_sb[:, kc * 72:(kc + 1) * 72],
                                wt_sb[:, k8 * 512:(k8 + 1) * 512],
                                start=(kc == 0), stop=False)
                    bia = wpool.tile([1, 512], bf16, tag="bia", bufs=2,
                                     name=f"dkb{nch}")
                    nc.sync.dma_start(bia[:], dkb.ap()[nch:nch + 1, :])
                    nc.tensor.matmul(ps_dw[:], onesb_sb[0:1, 0:72], bia[:],
                                     start=False, stop=True)
                    nc.vector.tensor_copy(dw_sb[:, nch * 512:(nch + 1) * 512],
                                          ps_dw[:])

                # pk slice for all samples: [8, 4096]
                pk_sb = apool.tile([8, OCS], bf16)
                for nch in range(8):
                    ps_pk = ps_a.tile([8, 512], f32, tag="psa", bufs=3,
                                      name=f"pspk{nch}")
                    pkt_sb = wpool.tile([128, 4 * 512], bf16, tag="pkw",
                                        bufs=2, name=f"pkt{nch}")
                    nc.sync.dma_start(
                        pkt_sb[:].rearrange("p (k c) -> p k c", k=4),
                        pkt.ap()[nch].rearrange("k p c -> p k c"))
                    for kc in range(4):
                        nc.tensor.matmul(
                            ps_pk[:],
                            sd_b[:, kc * 8:(kc + 1) * 8],
                            pkt_sb[:, kc * 512:(kc + 1) * 512],
                            start=(kc == 0), stop=False)
                    bia = wpool.tile([1, 512], bf16, tag="bia", bufs=2,
                                     name=f"pkb{nch}")
                    nc.sync.dma_start(bia[:], pkb.ap()[nch:nch + 1, :])
                    nc.tensor.matmul(ps_pk[:], onesb_sb[0:1, 0:8], bia[:],
                                     start=False, stop=True)
                    nc.vector.tensor_copy(pk_sb[:, nch * 512:(nch + 1) * 512],
                                          ps_pk[:])

                # pb for all samples: [oc(128) x 4, 8] (+ pwb_b bias)
                for occ in range(4):
                    ps_pb = ps_a.tile([128, 8], f32, tag="psa", bufs=3,
                                      name=f"pspb{occ}")
                    for kc in range(4):
                        nc.tensor.matmul(
                            ps_pb[:],
                            pbt_sb[:, kc * C + occ * 128: kc * C + occ * 128 + 128],
                            sd_b[:, kc * 8:(kc + 1) * 8],
                            start=(kc == 0), stop=(kc == 3))
                    nc.scalar.activation(pb_sb[:, occ * 8:(occ + 1) * 8], ps_pb[:],
                                         ACTF.Identity,
                                         bias=pwbb_sb[:, occ:occ + 1], scale=1.0)

                # ---------- A2A exchange ----------
                ain = a2a_in.ap()[:, 0:9 * OCS].rearrange(
                    "n (s f) -> n s f", s=9)
                for n in range(N_CORES):
                    nc.sync.dma_start(ain[n:n + 1, 0:9, :],
                                      dw_sb[n * 9:(n + 1) * 9, :])
                nc.sync.dma_start(a2a_in.ap()[:, 9 * OCS:10 * OCS], pk_sb[:])
                # pb for sample n into block n (all cores send the same)
                pbm = pb_sb[:].rearrange("p (o n) -> p o n", n=8)
                for n in range(N_CORES):
                    nc.scalar.dma_start(
                        a2a_in.ap()[n:n + 1, 10 * OCS:10 * OCS + 512]
                        .rearrange("n (o p) -> n p o", p=128),
                        pbm[:, :, n])
                nc.gpsimd.collective_compute(
                    "AllToAll", ALU.bypass,
                    replica_groups=[list(range(N_CORES))],
                    ins=[a2a_in.ap().opt()],
                    outs=[a2a_out.ap().opt()])

            # ============== post-A2A: E = pk @ dw (+ S), svt, conv ==========
            with tc.tile_pool(name="pe2", bufs=1) as e2pool, \
                 tc.tile_pool(name="opool", bufs=4) as opool:
                ob = a2a_out.ap()[:, 0:9 * OCS].rearrange(
                    "n (s j i) -> n j s i", s=9, j=64)
                pbv = e2pool.tile([128, 4], bf16)
                nc.sync.dma_start(
                    pbv[:],
                    a2a_out.ap()[0:1, 10 * OCS:10 * OCS + 512]
                    .rearrange("n (o p) -> n p o", p=128)[0])

                # ---------- stats finals (mu, 1/sigma) ----------
                mui = spool.tile([128, 1], f32)
                if norm:
                    part = spool.tile([128, 2], f32)
                    part_r = spool.tile([128, 2], f32r)
                    nc.vector.tensor_reduce(part[:, 0:1], acc[:, 0:32],
                                            axis=AX.X, op=ALU.add)
                    nc.vector.tensor_reduce(part[:, 1:2], acc[:, 32:64],
                                            axis=AX.X, op=ALU.add)
                    nc.vector.tensor_copy(part_r[:], part[:])
                    ps_tot = ps_a.tile([128, 2], f32, tag="psa", bufs=3,
                                       name="pstot")
                    nc.tensor.matmul(ps_tot[:], ones_sb[:], part_r[:],
                                     start=True, stop=True)
                    mu = spool.tile([128, 1], f32)
                    ex2 = spool.tile([128, 1], f32)
                    var = spool.tile([128, 1], f32)
                    std = spool.tile([128, 1], f32)
                    nc.scalar.mul(mu[:], ps_tot[:, 0:1], 1.0 / M_TOT)
                    nc.scalar.mul(ex2[:], ps_tot[:, 1:2], 1.0 / M_TOT)
                    nc.scalar.square(mui[:], mu[:])
                    nc.vector.tensor_sub(var[:], ex2[:], mui[:])
                    nc.vector.tensor_scalar_add(var[:], var[:], EPS)
                    nc.scalar.sqrt(std[:], var[:])
                    nc.vector.reciprocal(svt[:, 4:5], std[:])
                    nc.vector.tensor_mul(mui[:], mu[:], svt[:, 4:5])
                else:
                    nc.vector.memset(svt[:, 4:5], 1.0)
                    nc.vector.memset(mui[:], 0.0)

                # ---------- per group-pair: E, S, svt, then conv ----------
                ov = out.ap().rearrange("(b p) (h w) -> b p h w", p=128, w=W)
                n_chunks = (H + RPC - 1) // RPC
                tmp1 = spool.tile([128, 1], f32)
                for gp in range(4):
                    e_gp = epool.tile([128, 9 * 128], bf16, tag="egp", bufs=4,
                                      name=f"egp{gp}")
                    nc.sync.dma_start(e_gp[:], zeros_e.ap())
                    for q in (0, 1):
                        g = 2 * gp + q
                        dwt = e2pool.tile([128, 9 * 64], bf16, tag="dwt",
                                          bufs=2, name=f"dwt{g}")
                        nc.sync.dma_start(
                            dwt[0:64, :].rearrange("j (t i) -> j t i", t=9),
                            ob[g, :, 0:9, :])
                        pk_t = e2pool.tile([128, 64], bf16, tag="pk_t",
                                           bufs=2, name=f"pk_t{g}")
                        nc.sync.dma_start(
                            pk_t[0:64, :],
                            a2a_out.ap()[:, 9 * OCS:10 * OCS].rearrange(
                                "n (o j) -> n j o", j=64)[g])
                        dsum_f = e2pool.tile([128, 2], f32, tag="dsf", bufs=2,
                                             name=f"dsf{g}")
                        dsum_b = e2pool.tile([128, 2], bf16, tag="dsr", bufs=2,
                                             name=f"dsr{g}")
                        nc.vector.tensor_reduce(dsum_f[0:64, 0:1], dwt[0:64, :],
                                                axis=AX.X, op=ALU.add)
                        nc.vector.tensor_copy(dsum_f[0:64, 1:2],
                                              dsum_f[0:64, 0:1])
                        nc.vector.tensor_copy(dsum_b[0:64, :], dsum_f[0:64, :])
                        ps_s = ps_a.tile([128, 2], f32, tag="psa", bufs=3,
                                         name=f"pss{g}")
                        nc.tensor.matmul(ps_s[q * 64:q * 64 + 64, :],
                                         pk_t[0:64, :], dsum_b[0:64, :],
                                         start=True, stop=True,
                                         tile_position=(0, q * 64))
                        nc.scalar.copy(s_vec[q * 64:q * 64 + 64, gp:gp + 1],
                                       ps_s[q * 64:q * 64 + 64, 0:1])
                        for t in range(9):
                            ps_e = ps_c.tile([128, 64], f32, tag="pse", bufs=2,
                                             name=f"pse{g}_{t}")
                            nc.tensor.matmul(ps_e[q * 64:q * 64 + 64, :],
                                             dwt[0:64, t * 64:(t + 1) * 64],
                                             pk_t[0:64, :],
                                             start=True, stop=True,
                                             tile_position=(0, q * 64))
                            ecol = t * 128 + q * 64
                            nc.vector.tensor_copy(
                                e_gp[q * 64:q * 64 + 64, ecol:ecol + 64],
                                ps_e[q * 64:q * 64 + 64, :])
                    # svt for this group pair
                    nc.vector.tensor_mul(tmp1[:], s_vec[:, gp:gp + 1], mui[:])
                    nc.vector.tensor_sub(svt[:, gp:gp + 1],
                                         pbv[:, gp:gp + 1], tmp1[:])

                    # ---------- conv for this group pair ----------
                    xflat = xts[gp][:]
                    for ci in range(n_chunks):
                        r0 = ci * RPC
                        nrows = min(RPC, H - r0)
                        N = nrows * PW
                        if r0 + nrows >= H:
                            N -= 2
                        ps = ps_c.tile([128, NCH], f32, tag="ps", bufs=3,
                                       name=f"ps{gp}_{ci}")
                        for t in range(9):
                            i, j = t // 3, t % 3
                            off = (r0 + i) * PW + j
                            nc.tensor.matmul(ps[:, 0:N],
                                             e_gp[:, t * 128:t * 128 + 128],
                                             xflat[:, off:off + N],
                                             start=(t == 0), stop=(t == 8))
                        ot = opool.tile([128, NCH], f32, tag="ot",
                                        name=f"ot{gp}_{ci}")
                        nc.scalar.activation(ot[:, 0:N], ps[:, 0:N],
                                             ACTF.Identity,
                                             bias=svt[:, gp:gp + 1],
                                             scale=svt[:, 4:5])
                        osrc = ot[:, 0:nrows * PW].rearrange(
                            "p (r c) -> p r c", c=PW)
                        nc.sync.dma_start(ov[gp, :, r0:r0 + nrows, :],
                                          osrc[:, 0:nrows, 0:128])

    nc.compile()
    return nc


def _host_prep(style_encoding, dk_w, dk_b, pwk_w, pwk_b, pwb_w, pwb_b):
    """Build the per-core input shards (reshapes/transposes/casts only)."""
    f = np.float32
    bf = ml_dtypes.bfloat16
    st = np.asarray(style_encoding, f)                      # [8, 512, 4, 4]
    WTf = np.asarray(dk_w, f).reshape(32768, KM).T          # [2048, 32768] view
    PKTf = np.asarray(pwk_w, f).reshape(32768, 512).T       # [512, 32768] view
    PBT = np.ascontiguousarray(
        np.asarray(pwb_w, f).reshape(512, 512).T).reshape(4, 128, 512).astype(bf)

    S = np.empty((KM, 72), f)
    for kh in range(2):
        for kw in range(2):
            blk = st[:, :, kh:kh + 3, kw:kw + 3].reshape(8, 512, 9)
            S[kh * 2 + kw::4, :] = blk.transpose(1, 0, 2).reshape(512, 72)
    S = np.ascontiguousarray(S.reshape(16, 128, 72)).astype(bf)

    st_raw = np.ascontiguousarray(
        st.reshape(8, 4, 128, 16).transpose(1, 2, 0, 3))    # [4,128,8,16]
    pwbb = np.ascontiguousarray(
        np.asarray(pwb_b, f).reshape(4, 128).T)             # [128, 4]
    ones_r = np.ones((128, 128), f)
    ones_b = np.ones((1, 128), f).astype(bf)
    zeros_e = np.zeros((128, 9 * 128), bf)
    dkb_full = np.asarray(dk_b, f)
    pkb_full = np.asarray(pwk_b, f)

    shards = []
    for g in range(N_CORES):
        sl = slice(g * OCS, (g + 1) * OCS)
        # wt blocks: [16 kc, 8 nch, 128, 512] contiguous per (kc, nch)
        wtg = np.ascontiguousarray(WTf[:, sl]).reshape(16, 128, 8, 512)
        wtg = np.ascontiguousarray(wtg.transpose(2, 0, 1, 3)).astype(bf)
        pktg = np.ascontiguousarray(PKTf[:, sl]).reshape(4, 128, 8, 512)
        pktg = np.ascontiguousarray(pktg.transpose(2, 0, 1, 3)).astype(bf)
        shards.append(dict(
            wt=wtg, pkt=pktg, pbt=PBT, s_im=S, st_raw=st_raw,
            dkb=np.ascontiguousarray(dkb_full[sl]).reshape(8, 512).astype(bf),
            pkb=np.ascontiguousarray(pkb_full[sl]).reshape(8, 512).astype(bf),
            pwbb=pwbb, ones_r=ones_r, ones_b=ones_b, zeros_e=zeros_e,
        ))
    return shards


def kernel(style_encoding, predicted, dk_w, dk_b, pwk_w, pwk_b, pwb_w, pwb_b,
           norm=True, **_ignored):
    from concourse import bass_utils

    norm = bool(norm)
    key = ("nc", norm)
    if key not in _CACHE:
        _CACHE[key] = _build(norm)
    nc = _CACHE[key]

    pred = np.ascontiguousarray(np.asarray(predicted, np.float32))
    shards = _host_prep(style_encoding, dk_w, dk_b, pwk_w, pwk_b,
                        pwb_w, pwb_b)
    in_maps = []
    for g in range(N_CORES):
        m = dict(shards[g])
        m["xin"] = pred[g].reshape(C, H * W)
        in_maps.append(m)

    res = bass_utils.run_bass_kernel_spmd(nc, in_maps,
                                          core_ids=list(range(N_CORES)))
    out = np.stack([res.results[g]["out"].reshape(C, H, W)
                    for g in range(N_CORES)])
    return out.astype(np.float32)

